# revision 5
# baseline (speedup 1.0000x reference)
"""Weighted BCE loss (nn_BCELoss_with_weight) on 8 Trainium2 NeuronCores.

Reference computes:
    log_p   = max(log(pred), -100)            # clamp never binds: pred in [1e-4, 1-1e-4]
    log_1mp = max(log1p(-pred), -100)
    bce     = -(true*log_p + (1-true)*log_1mp)    # [B,C,D,H,W] = [2,16,64,128,128]
    per_class = mean(bce, axes=(0,2,3,4))         # [C]
    out = sum(weight*per_class) / sum(weight)     # scalar

Sharding: D=64 split into 8 slices of 8 (data parallel). Per core the shard
[2,16,8,128,128] is viewed as [B=2, (C,Dl)=128, H*W=16384]: partition p holds
class c=p//8 only, so the per-class weight is a per-partition scalar.

Per core on device, with u=ln(p), v=ln(1-p), w~=bf16(weight):
    term = t*u + (1-t)*v = t*(u-v) + v
    DMA : pred f32 on the sync HWDGE ring (sequencer issues nothing else, so
          issue never blocks behind compute); true on gpsimd SWDGE with inline
          f32->bf16 cast.
    ACT : u = Ln(p) [bf16], v = Ln(-p+1) [bf16]
    DVE : d = u - v (bf16 TT 2x);  m = t*d (bf16 TT 2x)
    PE  : psum[1,512] += wf[128,1].T @ v_chunk  and  += wf.T @ m_chunk
          (both streams weighted by wf and accumulated in one f32 PSUM bank)
    out[1,1] = sum(psum)   -- single 4-byte output, one DMA descriptor
          (a [128,1] output would be 128 4-byte HBM read-modify-writes whose
          completion receipts serialize ~6us on the SDMA engines)
Host: result = -(sum_cores out) / (M * sum(w~)), M = B*D*H*W. Using the
bf16-rounded weights consistently in numerator and denominator makes this the
exact weighted mean of per-class BCE with weights w~; per-class means are
~equal so the w->w~ rounding perturbs the result by ~1e-5 relative.
"""

import numpy as np

N_CORES = 8
B, C, D, H, W = 2, 16, 64, 128, 128
HW = H * W            # 16384 free elems per (b, partition)
P = 128               # (C=16) x (D_local=8) partitions
D_LOCAL = D // N_CORES
MM_N = 512            # one PSUM bank of f32

# Per-b DMA segment plans: mids big for DMA/ACT efficiency, small tail so the
# last chunk's LN->DVE->PE chain after the final byte is short.
SEGS_B0 = (2048, 4096, 4096, 4096, 2048)
SEGS_B1 = (4096, 4096, 4096, 2048, 1024, 512, 512)


def build_bass_kernel(segs_b0=SEGS_B0, segs_b1=SEGS_B1,
                      pin_bufs=4, tin_bufs=4, uv_bufs=6, sub=4096,
                      alternate=True, direct_reduce=True):
    """Build the per-core Bass/Tile kernel.

    Inputs  : pred, true [B, 128, free] f32 (shard, class*d_local on axis 1)
              wf [128, 1] bf16 (per-partition class weight)
    Outputs : out_m [1, 1] f32 = sum_p wf[p] * sum_e (t*(u-v) + v)[p, e]
    """
    import concourse.bacc as bacc
    import concourse.mybir as mybir
    import concourse.tile as tile

    f32 = mybir.dt.float32
    bf16 = mybir.dt.bfloat16
    AF = mybir.ActivationFunctionType

    segs_per_b = [list(segs_b0), list(segs_b1)]
    for segs in segs_per_b:
        assert sum(segs) == HW, segs
    plan = []                       # (b, offset, seg)
    total_mm = 0
    for b in range(B):
        off = 0
        for seg in segs_per_b[b]:
            plan.append((b, off, seg))
            total_mm += 2 * max(1, seg // MM_N)
            off += seg

    nc = bacc.Bacc("TRN2", target_bir_lowering=False, debug=False,
                   num_devices=N_CORES)
    pred_d = nc.dram_tensor("pred", [B, P, HW], f32, kind="ExternalInput")
    true_d = nc.dram_tensor("true", [B, P, HW], f32, kind="ExternalInput")
    wf_d = nc.dram_tensor("wf", [P, 1], bf16, kind="ExternalInput")
    outm_d = nc.dram_tensor("out_m", [1, 1], f32, kind="ExternalOutput")

    with tile.TileContext(nc) as tc:
        with (
            tc.tile_pool(name="pin", bufs=pin_bufs) as pin,
            tc.tile_pool(name="tin", bufs=tin_bufs) as tin,
            tc.tile_pool(name="uv", bufs=uv_bufs) as uvp,
            tc.tile_pool(name="small", bufs=1) as small,
            tc.tile_pool(name="psum", bufs=1, space="PSUM") as psump,
        ):
            wf_t = small.tile([P, 1], bf16, tag="wf")
            nc.gpsimd.dma_start(wf_t[:], wf_d[:])
            acc = psump.tile([1, MM_N], f32, tag="acc")
            # warm up the Ln table set so the first real ACTIVATE doesn't pay
            # the ~2.7us ACT_TABLE_LOAD after its data lands. Input comes from
            # a memset (not the wf DMA) so the warm-up never blocks the ACT
            # FIFO behind a DMA-completion semaphore.
            warm_in = small.tile([P, 1], f32, tag="warm_in")
            nc.vector.memset(warm_in[:], 1.0)
            warm = small.tile([P, 1], bf16, tag="warm")
            nc.scalar.activation(warm[:], warm_in[:], AF.Ln, bias=1.0,
                                 scale=1.0)

            mm_i = 0
            for pi, (b, off, seg) in enumerate(plan):
                p_t = pin.tile([P, seg], f32, tag="p")
                sl = slice(off, off + seg)
                # Alternate which queue carries pred vs true each seg so the
                # two streams advance in lockstep (neither queue's packet mix
                # lets one stream starve the other): even segs put pred on the
                # sync HWDGE ring and true on the gpsimd SWDGE ring (with
                # inline f32->bf16 cast); odd segs swap queues, so that seg's
                # true tile stays f32 (HWDGE cannot cast).
                if alternate and pi % 2:
                    t_t = tin.tile([P, seg], f32, tag="t")
                    nc.gpsimd.dma_start(p_t[:], pred_d[b, :, sl])
                    nc.sync.dma_start(t_t[:], true_d[b, :, sl])
                else:
                    t_t = tin.tile([P, seg], bf16, tag="t")
                    nc.sync.dma_start(p_t[:], pred_d[b, :, sl])
                    nc.gpsimd.dma_start(t_t[:], true_d[b, :, sl])
                s_off = 0
                while s_off < seg:
                    s_sz = min(sub, seg - s_off)
                    ss = slice(s_off, s_off + s_sz)
                    u = uvp.tile([P, s_sz], bf16, tag="u")
                    v = uvp.tile([P, s_sz], bf16, tag="v")
                    nc.scalar.activation(u[:], p_t[:, ss], AF.Ln,
                                         bias=0.0, scale=1.0)
                    nc.scalar.activation(v[:], p_t[:, ss], AF.Ln,
                                         bias=1.0, scale=-1.0)
                    # acc += wf.T @ v (v is ready first; PE runs these while
                    # DVE forms m), then acc += wf.T @ m
                    for q in range(max(1, s_sz // MM_N)):
                        qq = slice(q * MM_N, min((q + 1) * MM_N, s_sz))
                        nc.tensor.matmul(acc[:, 0:qq.stop - qq.start],
                                         wf_t[:], v[:, qq],
                                         start=(mm_i == 0),
                                         stop=(mm_i == total_mm - 1))
                        mm_i += 1
                    # u <- d = u - v ; u <- m = t * d   (bf16 2x TT)
                    nc.vector.tensor_sub(u[:], u[:], v[:])
                    nc.vector.tensor_mul(u[:], t_t[:, ss], u[:])
                    for q in range(max(1, s_sz // MM_N)):
                        qq = slice(q * MM_N, min((q + 1) * MM_N, s_sz))
                        nc.tensor.matmul(acc[:, 0:qq.stop - qq.start],
                                         wf_t[:], u[:, qq],
                                         start=(mm_i == 0),
                                         stop=(mm_i == total_mm - 1))
                        mm_i += 1
                    s_off += s_sz
            assert mm_i == total_mm

            outm_t = small.tile([1, 1], f32, tag="outm")
            if direct_reduce:
                nc.vector.reduce_sum(outm_t[:], acc[:],
                                     axis=mybir.AxisListType.X)
            else:
                accm_sb = small.tile([1, MM_N], f32, tag="accm_sb")
                nc.vector.tensor_copy(accm_sb[:], acc[:])
                nc.vector.reduce_sum(outm_t[:], accm_sb[:],
                                     axis=mybir.AxisListType.X)
            nc.sync.dma_start(outm_d[:], outm_t[:])

    nc.compile()
    return nc


_NC_CACHE = {}


def _get_nc():
    if "nc" not in _NC_CACHE:
        import json
        import os

        opts = json.loads(os.environ.get("KERNEL_OPTS", "{}"))
        for k in ("segs_b0", "segs_b1"):
            if k in opts:
                opts[k] = tuple(opts[k])
        _NC_CACHE["nc"] = build_bass_kernel(**opts)
    return _NC_CACHE["nc"]


def _bf16_round(x):
    """Round f32 array to bf16 values (kept in f32 representation)."""
    xi = np.asarray(x, dtype=np.float32).view(np.uint32)
    rounded = ((xi + 0x7FFF + ((xi >> 16) & 1)) & 0xFFFF0000).astype(np.uint32)
    return rounded.view(np.float32)


def shard_inputs(pred, true, weight):
    """Full [B,C,D,H,W] -> per-core in_maps."""
    import ml_dtypes

    wtile = np.repeat(np.asarray(weight, np.float32), D_LOCAL).reshape(P, 1)
    wf = wtile.astype(ml_dtypes.bfloat16)
    in_maps = []
    for i in range(N_CORES):
        d0 = i * D_LOCAL
        ps = np.ascontiguousarray(
            pred[:, :, d0:d0 + D_LOCAL].reshape(B, P, HW))
        ts = np.ascontiguousarray(
            true[:, :, d0:d0 + D_LOCAL].reshape(B, P, HW))
        in_maps.append({"pred": ps, "true": ts, "wf": wf})
    return in_maps


def combine(out_ms, weight):
    """out_ms [n_cores] scalars; weight [16] f32."""
    wt = _bf16_round(np.repeat(np.asarray(weight, np.float32), D_LOCAL))
    m = float(B * D * H * W)
    w_sum = wt.astype(np.float64)[::D_LOCAL].sum()   # sum of bf16 class weights
    total = float(np.asarray(out_ms, np.float64).sum())
    return np.float32(-total / (m * w_sum))


def kernel(pred, true, weight, _trace=False):
    from concourse.bass_utils import run_bass_kernel_spmd

    nc = _get_nc()
    in_maps = shard_inputs(np.asarray(pred), np.asarray(true), weight)
    res = run_bass_kernel_spmd(nc, in_maps, core_ids=list(range(N_CORES)),
                               trace=_trace)
    out_ms = [r["out_m"][0, 0] for r in res.results]
    out = combine(out_ms, weight)
    if _trace:
        return out, res
    return out


# revision 6
# speedup vs baseline: 1.0772x; 1.0772x over previous
"""Weighted BCE loss (nn_BCELoss_with_weight) on 8 Trainium2 NeuronCores.

Reference computes:
    log_p   = max(log(pred), -100)            # clamp never binds: pred in [1e-4, 1-1e-4]
    log_1mp = max(log1p(-pred), -100)
    bce     = -(true*log_p + (1-true)*log_1mp)    # [B,C,D,H,W] = [2,16,64,128,128]
    per_class = mean(bce, axes=(0,2,3,4))         # [C]
    out = sum(weight*per_class) / sum(weight)     # scalar

Sharding: D=64 split into 8 slices of 8 (data parallel). Per core the shard
[2,16,8,128,128] is viewed as [B=2, (C,Dl)=128, H*W=16384]: partition p holds
class c=p//8 only, so the per-class weight is a per-partition scalar.

Per core on device, with u=ln(p), v=ln(1-p), w~=bf16(weight):
    term = t*u + (1-t)*v = t*(u-v) + v
    DMA : pred f32 on the sync HWDGE ring (sequencer issues nothing else, so
          issue never blocks behind compute); true on gpsimd SWDGE with inline
          f32->bf16 cast.
    ACT : u = Ln(p) [bf16], v = Ln(-p+1) [bf16]
    DVE : d = u - v (bf16 TT 2x);  m = t*d (bf16 TT 2x)
    PE  : psum[1,512] += wf[128,1].T @ v_chunk  and  += wf.T @ m_chunk
          (both streams weighted by wf and accumulated in one f32 PSUM bank)
    out[1,1] = sum(psum)   -- single 4-byte output, one DMA descriptor
          (a [128,1] output would be 128 4-byte HBM read-modify-writes whose
          completion receipts serialize ~6us on the SDMA engines)
Host: result = -(sum_cores out) / (M * sum(w~)), M = B*D*H*W. Using the
bf16-rounded weights consistently in numerator and denominator makes this the
exact weighted mean of per-class BCE with weights w~; per-class means are
~equal so the w->w~ rounding perturbs the result by ~1e-5 relative.
"""

import numpy as np

N_CORES = 8
B, C, D, H, W = 2, 16, 64, 128, 128
HW = H * W            # 16384 free elems per (b, partition)
P = 128               # (C=16) x (D_local=8) partitions
D_LOCAL = D // N_CORES
MM_N = 512            # one PSUM bank of f32

# Per-b DMA segment plans: mids big for DMA/ACT efficiency, small tail so the
# last chunk's LN->DVE->PE chain after the final byte is short.
SEGS_B0 = (1024, 2048, 2048, 2048, 2048, 2048, 2048, 2048, 1024)
SEGS_B1 = (2048, 2048, 2048, 2048, 2048, 2048, 2048, 1024, 512, 512)


def build_bass_kernel(segs_b0=SEGS_B0, segs_b1=SEGS_B1,
                      pin_bufs=10, tin_bufs=10, uv_bufs=6, sub=2048,
                      alternate=False, direct_reduce=True):
    """Build the per-core Bass/Tile kernel.

    Inputs  : pred, true [B, 128, free] f32 (shard, class*d_local on axis 1)
              wf [128, 1] bf16 (per-partition class weight)
    Outputs : out_m [1, 1] f32 = sum_p wf[p] * sum_e (t*(u-v) + v)[p, e]
    """
    import concourse.bacc as bacc
    import concourse.mybir as mybir
    import concourse.tile as tile

    f32 = mybir.dt.float32
    bf16 = mybir.dt.bfloat16
    AF = mybir.ActivationFunctionType

    segs_per_b = [list(segs_b0), list(segs_b1)]
    for segs in segs_per_b:
        assert sum(segs) == HW, segs
    plan = []                       # (b, offset, seg)
    total_mm = 0
    for b in range(B):
        off = 0
        for seg in segs_per_b[b]:
            plan.append((b, off, seg))
            total_mm += 2 * max(1, seg // MM_N)
            off += seg

    nc = bacc.Bacc("TRN2", target_bir_lowering=False, debug=False,
                   num_devices=N_CORES)
    pred_d = nc.dram_tensor("pred", [B, P, HW], f32, kind="ExternalInput")
    true_d = nc.dram_tensor("true", [B, P, HW], f32, kind="ExternalInput")
    wf_d = nc.dram_tensor("wf", [P, 1], bf16, kind="ExternalInput")
    outm_d = nc.dram_tensor("out_m", [1, 1], f32, kind="ExternalOutput")

    with tile.TileContext(nc) as tc:
        with (
            tc.tile_pool(name="pin", bufs=pin_bufs) as pin,
            tc.tile_pool(name="tin", bufs=tin_bufs) as tin,
            tc.tile_pool(name="uv", bufs=uv_bufs) as uvp,
            tc.tile_pool(name="small", bufs=1) as small,
            tc.tile_pool(name="psum", bufs=1, space="PSUM") as psump,
        ):
            wf_t = small.tile([P, 1], bf16, tag="wf")
            nc.gpsimd.dma_start(wf_t[:], wf_d[:])
            acc = psump.tile([1, MM_N], f32, tag="acc")
            # warm up the Ln table set so the first real ACTIVATE doesn't pay
            # the ~2.7us ACT_TABLE_LOAD after its data lands. Input comes from
            # a memset (not the wf DMA) so the warm-up never blocks the ACT
            # FIFO behind a DMA-completion semaphore.
            warm_in = small.tile([P, 1], f32, tag="warm_in")
            nc.vector.memset(warm_in[:], 1.0)
            warm = small.tile([P, 1], bf16, tag="warm")
            nc.scalar.activation(warm[:], warm_in[:], AF.Ln, bias=1.0,
                                 scale=1.0)

            mm_i = 0
            for pi, (b, off, seg) in enumerate(plan):
                p_t = pin.tile([P, seg], f32, tag="p")
                sl = slice(off, off + seg)
                # Alternate which queue carries pred vs true each seg so the
                # two streams advance in lockstep (neither queue's packet mix
                # lets one stream starve the other): even segs put pred on the
                # sync HWDGE ring and true on the gpsimd SWDGE ring (with
                # inline f32->bf16 cast); odd segs swap queues, so that seg's
                # true tile stays f32 (HWDGE cannot cast).
                if alternate and pi % 2:
                    t_t = tin.tile([P, seg], f32, tag="t")
                    nc.gpsimd.dma_start(p_t[:], pred_d[b, :, sl])
                    nc.sync.dma_start(t_t[:], true_d[b, :, sl])
                else:
                    t_t = tin.tile([P, seg], bf16, tag="t")
                    nc.sync.dma_start(p_t[:], pred_d[b, :, sl])
                    nc.gpsimd.dma_start(t_t[:], true_d[b, :, sl])
                s_off = 0
                while s_off < seg:
                    s_sz = min(sub, seg - s_off)
                    ss = slice(s_off, s_off + s_sz)
                    u = uvp.tile([P, s_sz], bf16, tag="u")
                    v = uvp.tile([P, s_sz], bf16, tag="v")
                    nc.scalar.activation(u[:], p_t[:, ss], AF.Ln,
                                         bias=0.0, scale=1.0)
                    nc.scalar.activation(v[:], p_t[:, ss], AF.Ln,
                                         bias=1.0, scale=-1.0)
                    # acc += wf.T @ v (v is ready first; PE runs these while
                    # DVE forms m), then acc += wf.T @ m
                    for q in range(max(1, s_sz // MM_N)):
                        qq = slice(q * MM_N, min((q + 1) * MM_N, s_sz))
                        nc.tensor.matmul(acc[:, 0:qq.stop - qq.start],
                                         wf_t[:], v[:, qq],
                                         start=(mm_i == 0),
                                         stop=(mm_i == total_mm - 1))
                        mm_i += 1
                    # u <- d = u - v ; u <- m = t * d   (bf16 2x TT)
                    nc.vector.tensor_sub(u[:], u[:], v[:])
                    nc.vector.tensor_mul(u[:], t_t[:, ss], u[:])
                    for q in range(max(1, s_sz // MM_N)):
                        qq = slice(q * MM_N, min((q + 1) * MM_N, s_sz))
                        nc.tensor.matmul(acc[:, 0:qq.stop - qq.start],
                                         wf_t[:], u[:, qq],
                                         start=(mm_i == 0),
                                         stop=(mm_i == total_mm - 1))
                        mm_i += 1
                    s_off += s_sz
            assert mm_i == total_mm

            outm_t = small.tile([1, 1], f32, tag="outm")
            if direct_reduce:
                nc.vector.reduce_sum(outm_t[:], acc[:],
                                     axis=mybir.AxisListType.X)
            else:
                accm_sb = small.tile([1, MM_N], f32, tag="accm_sb")
                nc.vector.tensor_copy(accm_sb[:], acc[:])
                nc.vector.reduce_sum(outm_t[:], accm_sb[:],
                                     axis=mybir.AxisListType.X)
            nc.sync.dma_start(outm_d[:], outm_t[:])

    nc.compile()
    return nc


_NC_CACHE = {}


def _get_nc():
    if "nc" not in _NC_CACHE:
        import json
        import os

        opts = json.loads(os.environ.get("KERNEL_OPTS", "{}"))
        for k in ("segs_b0", "segs_b1"):
            if k in opts:
                opts[k] = tuple(opts[k])
        _NC_CACHE["nc"] = build_bass_kernel(**opts)
    return _NC_CACHE["nc"]


def _bf16_round(x):
    """Round f32 array to bf16 values (kept in f32 representation)."""
    xi = np.asarray(x, dtype=np.float32).view(np.uint32)
    rounded = ((xi + 0x7FFF + ((xi >> 16) & 1)) & 0xFFFF0000).astype(np.uint32)
    return rounded.view(np.float32)


def shard_inputs(pred, true, weight):
    """Full [B,C,D,H,W] -> per-core in_maps."""
    import ml_dtypes

    wtile = np.repeat(np.asarray(weight, np.float32), D_LOCAL).reshape(P, 1)
    wf = wtile.astype(ml_dtypes.bfloat16)
    in_maps = []
    for i in range(N_CORES):
        d0 = i * D_LOCAL
        ps = np.ascontiguousarray(
            pred[:, :, d0:d0 + D_LOCAL].reshape(B, P, HW))
        ts = np.ascontiguousarray(
            true[:, :, d0:d0 + D_LOCAL].reshape(B, P, HW))
        in_maps.append({"pred": ps, "true": ts, "wf": wf})
    return in_maps


def combine(out_ms, weight):
    """out_ms [n_cores] scalars; weight [16] f32."""
    wt = _bf16_round(np.repeat(np.asarray(weight, np.float32), D_LOCAL))
    m = float(B * D * H * W)
    w_sum = wt.astype(np.float64)[::D_LOCAL].sum()   # sum of bf16 class weights
    total = float(np.asarray(out_ms, np.float64).sum())
    return np.float32(-total / (m * w_sum))


def kernel(pred, true, weight, _trace=False):
    from concourse.bass_utils import run_bass_kernel_spmd

    nc = _get_nc()
    in_maps = shard_inputs(np.asarray(pred), np.asarray(true), weight)
    res = run_bass_kernel_spmd(nc, in_maps, core_ids=list(range(N_CORES)),
                               trace=_trace)
    out_ms = [r["out_m"][0, 0] for r in res.results]
    out = combine(out_ms, weight)
    if _trace:
        return out, res
    return out


# revision 9
# speedup vs baseline: 1.2528x; 1.1630x over previous
"""Weighted BCE loss (nn_BCELoss_with_weight) on 8 Trainium2 NeuronCores.

Reference computes:
    log_p   = max(log(pred), -100)            # clamp never binds: pred in [1e-4, 1-1e-4]
    log_1mp = max(log1p(-pred), -100)
    bce     = -(true*log_p + (1-true)*log_1mp)    # [B,C,D,H,W] = [2,16,64,128,128]
    per_class = mean(bce, axes=(0,2,3,4))         # [C]
    out = sum(weight*per_class) / sum(weight)     # scalar

Sharding: D=64 split into 8 slices of 8 (data parallel). Per core the shard
[2,16,8,128,128] is viewed as [B=2, (C,Dl)=128, H*W=16384]: partition p holds
class c=p//8 only, so the per-class weight is a per-partition scalar.

Per core on device, with u=ln(p), v=ln(1-p), w~=bf16(weight):
    term = t*u + (1-t)*v = t*(u-v) + v
    DMA : pred f32 on the sync HWDGE ring (sequencer issues nothing else, so
          issue never blocks behind compute); true on gpsimd SWDGE with inline
          f32->bf16 cast.
    ACT : u = Ln(p) [bf16], v = Ln(-p+1) [bf16]
    DVE : d = u - v (bf16 TT 2x);  m = t*d (bf16 TT 2x)
    PE  : psum[1,512] += wf[128,1].T @ v_chunk  and  += wf.T @ m_chunk
          (both streams weighted by wf and accumulated in one f32 PSUM bank)
    out[1,1] = sum(psum)   -- single 4-byte output, one DMA descriptor
          (a [128,1] output would be 128 4-byte HBM read-modify-writes whose
          completion receipts serialize ~6us on the SDMA engines)
Host: result = -(sum_cores out) / (M * sum(w~)), M = B*D*H*W. Using the
bf16-rounded weights consistently in numerator and denominator makes this the
exact weighted mean of per-class BCE with weights w~; per-class means are
~equal so the w->w~ rounding perturbs the result by ~1e-5 relative.
"""

import numpy as np

N_CORES = 8
B, C, D, H, W = 2, 16, 64, 128, 128
HW = H * W            # 16384 free elems per (b, partition)
P = 128               # (C=16) x (D_local=8) partitions
D_LOCAL = D // N_CORES
MM_N = 512            # one PSUM bank of f32

# Per-b DMA segment plans: mids big for DMA/ACT efficiency, small tail so the
# last chunk's LN->DVE->PE chain after the final byte is short.
SEGS_B0 = (1024, 2048, 2048, 2048, 2048, 2048, 2048, 2048, 1024)
SEGS_B1 = (2048, 2048, 2048, 2048, 2048, 2048, 2048, 1024, 512, 512)


def build_bass_kernel(segs_b0=SEGS_B0, segs_b1=SEGS_B1,
                      pin_bufs=10, tin_bufs=10, uv_bufs=5, m_bufs=4,
                      sub=2048, mul_lag=2, alternate=False,
                      direct_reduce=True):
    """Build the per-core Bass/Tile kernel.

    Inputs  : pred, true [B, 128, free] f32 (shard, class*d_local on axis 1)
              wf [128, 1] bf16 (per-partition class weight)
    Outputs : out_m [1, 1] f32 = sum_p wf[p] * sum_e (t*(u-v) + v)[p, e]
    """
    import concourse.bacc as bacc
    import concourse.mybir as mybir
    import concourse.tile as tile

    f32 = mybir.dt.float32
    bf16 = mybir.dt.bfloat16
    AF = mybir.ActivationFunctionType

    segs_per_b = [list(segs_b0), list(segs_b1)]
    for segs in segs_per_b:
        assert sum(segs) == HW, segs
    plan = []                       # (b, offset, seg)
    total_mm = 0
    for b in range(B):
        off = 0
        for seg in segs_per_b[b]:
            plan.append((b, off, seg))
            total_mm += 2 * max(1, seg // MM_N)
            off += seg

    nc = bacc.Bacc("TRN2", target_bir_lowering=False, debug=False,
                   num_devices=N_CORES)
    pred_d = nc.dram_tensor("pred", [B, P, HW], f32, kind="ExternalInput")
    true_d = nc.dram_tensor("true", [B, P, HW], f32, kind="ExternalInput")
    wf_d = nc.dram_tensor("wf", [P, 1], bf16, kind="ExternalInput")
    outm_d = nc.dram_tensor("out_m", [1, 1], f32, kind="ExternalOutput")

    with tile.TileContext(nc) as tc:
        with (
            tc.tile_pool(name="pin", bufs=pin_bufs) as pin,
            tc.tile_pool(name="tin", bufs=tin_bufs) as tin,
            tc.tile_pool(name="uv", bufs=uv_bufs) as uvp,
            tc.tile_pool(name="mp", bufs=m_bufs) as mp,
            tc.tile_pool(name="small", bufs=1) as small,
            tc.tile_pool(name="psum", bufs=1, space="PSUM") as psump,
        ):
            wf_t = small.tile([P, 1], bf16, tag="wf")
            nc.gpsimd.dma_start(wf_t[:], wf_d[:])
            acc = psump.tile([1, MM_N], f32, tag="acc")
            # warm up the Ln table set so the first real ACTIVATE doesn't pay
            # the ~2.7us ACT_TABLE_LOAD after its data lands. Input comes from
            # a memset (not the wf DMA) so the warm-up never blocks the ACT
            # FIFO behind a DMA-completion semaphore.
            warm_in = small.tile([P, 1], f32, tag="warm_in")
            nc.vector.memset(warm_in[:], 1.0)
            warm = small.tile([P, 1], bf16, tag="warm")
            nc.scalar.activation(warm[:], warm_in[:], AF.Ln, bias=1.0,
                                 scale=1.0)

            mm_i = 0
            # Pipeline the t-dependent DVE muls `mul_lag` sub-chunks behind
            # the subs: a mul waiting on its true-chunk DMA must not
            # head-of-line-block the next sub in DVE's FIFO (that stall
            # cascades: uv recycling -> ACT -> pin recycling -> pred DMA).
            pending = []        # (m_tile, t_tile, t_slice, width)

            def mm(src, w):
                nonlocal mm_i
                for q in range(max(1, w // MM_N)):
                    qq = slice(q * MM_N, min((q + 1) * MM_N, w))
                    nc.tensor.matmul(acc[:, 0:qq.stop - qq.start],
                                     wf_t[:], src[:, qq],
                                     start=(mm_i == 0),
                                     stop=(mm_i == total_mm - 1))
                    mm_i += 1

            def flush_one():
                m_t, t_t, tss, w = pending.pop(0)
                nc.vector.tensor_mul(m_t[:], t_t[:, tss], m_t[:])
                mm(m_t, w)

            for (b, off, seg) in plan:
                p_t = pin.tile([P, seg], f32, tag="p")
                t_t = tin.tile([P, seg], bf16, tag="t")
                sl = slice(off, off + seg)
                nc.sync.dma_start(p_t[:], pred_d[b, :, sl])
                # f32 -> bf16 cast inline (SWDGE-only feature)
                nc.gpsimd.dma_start(t_t[:], true_d[b, :, sl])
                s_off = 0
                while s_off < seg:
                    s_sz = min(sub, seg - s_off)
                    ss = slice(s_off, s_off + s_sz)
                    u = uvp.tile([P, s_sz], bf16, tag="u")
                    v = uvp.tile([P, s_sz], bf16, tag="v")
                    nc.scalar.activation(u[:], p_t[:, ss], AF.Ln,
                                         bias=0.0, scale=1.0)
                    nc.scalar.activation(v[:], p_t[:, ss], AF.Ln,
                                         bias=1.0, scale=-1.0)
                    # acc += wf.T @ v (v is ready first; PE runs these while
                    # DVE forms m), later acc += wf.T @ m
                    mm(v, s_sz)
                    # d = u - v into a separate tile so u/v recycle without
                    # waiting on the t-gated mul
                    m_t = mp.tile([P, s_sz], bf16, tag="m")
                    nc.vector.tensor_sub(m_t[:], u[:], v[:])
                    pending.append((m_t, t_t, ss, s_sz))
                    while len(pending) > mul_lag:
                        flush_one()
                    s_off += s_sz
            while pending:
                flush_one()
            assert mm_i == total_mm

            outm_t = small.tile([1, 1], f32, tag="outm")
            if direct_reduce:
                nc.vector.reduce_sum(outm_t[:], acc[:],
                                     axis=mybir.AxisListType.X)
            else:
                accm_sb = small.tile([1, MM_N], f32, tag="accm_sb")
                nc.vector.tensor_copy(accm_sb[:], acc[:])
                nc.vector.reduce_sum(outm_t[:], accm_sb[:],
                                     axis=mybir.AxisListType.X)
            nc.sync.dma_start(outm_d[:], outm_t[:])

    nc.compile()
    return nc


_NC_CACHE = {}


def _get_nc():
    if "nc" not in _NC_CACHE:
        import json
        import os

        opts = json.loads(os.environ.get("KERNEL_OPTS", "{}"))
        for k in ("segs_b0", "segs_b1"):
            if k in opts:
                opts[k] = tuple(opts[k])
        _NC_CACHE["nc"] = build_bass_kernel(**opts)
    return _NC_CACHE["nc"]


def _bf16_round(x):
    """Round f32 array to bf16 values (kept in f32 representation)."""
    xi = np.asarray(x, dtype=np.float32).view(np.uint32)
    rounded = ((xi + 0x7FFF + ((xi >> 16) & 1)) & 0xFFFF0000).astype(np.uint32)
    return rounded.view(np.float32)


def shard_inputs(pred, true, weight):
    """Full [B,C,D,H,W] -> per-core in_maps."""
    import ml_dtypes

    wtile = np.repeat(np.asarray(weight, np.float32), D_LOCAL).reshape(P, 1)
    wf = wtile.astype(ml_dtypes.bfloat16)
    in_maps = []
    for i in range(N_CORES):
        d0 = i * D_LOCAL
        ps = np.ascontiguousarray(
            pred[:, :, d0:d0 + D_LOCAL].reshape(B, P, HW))
        ts = np.ascontiguousarray(
            true[:, :, d0:d0 + D_LOCAL].reshape(B, P, HW))
        in_maps.append({"pred": ps, "true": ts, "wf": wf})
    return in_maps


def combine(out_ms, weight):
    """out_ms [n_cores] scalars; weight [16] f32."""
    wt = _bf16_round(np.repeat(np.asarray(weight, np.float32), D_LOCAL))
    m = float(B * D * H * W)
    w_sum = wt.astype(np.float64)[::D_LOCAL].sum()   # sum of bf16 class weights
    total = float(np.asarray(out_ms, np.float64).sum())
    return np.float32(-total / (m * w_sum))


def kernel(pred, true, weight, _trace=False):
    from concourse.bass_utils import run_bass_kernel_spmd

    nc = _get_nc()
    in_maps = shard_inputs(np.asarray(pred), np.asarray(true), weight)
    res = run_bass_kernel_spmd(nc, in_maps, core_ids=list(range(N_CORES)),
                               trace=_trace)
    out_ms = [r["out_m"][0, 0] for r in res.results]
    out = combine(out_ms, weight)
    if _trace:
        return out, res
    return out
